# revision 1
# baseline (speedup 1.0000x reference)
"""Trainium2 Bass kernel for 16-head MHA (B=2, S=2048, D=1024), fp32 I/O.

Sharding: tensor-parallel by heads across 8 NeuronCores. Core c owns heads
2c, 2c+1 (a 128-wide slice of the QKV projection output and of Wo's input
dim). Each core computes its head group's full attention plus a partial
output projection; the host sums the 8 partials.

Per-core dataflow (feature-major so the PE contraction dim is always the
SBUF partition dim; the host pre-transposes q/k/v and weights, and casts
activations + QKV weights to bf16):
  QT/KT [128, 4096] bf16, VT fp32r   = W_c @ x.T   (bf16 MMs, fp32 accum)
  V+ tiles [128j, 65] bf16           = VT j-chunks PE-transposed + ones col
  per (b, i-window 1024) window, phase 1 (jc = 0..15, h = 0..1):
    S.T [128j, 1024i] = KT_h_jc.T @ QT_h   bf16 scores, transposed; the two
                                           heads land in opposite PE array
                                           row-halves (row-tiled)
    E [128j, 1024i] = exp(S.T / 8)         ACT, PSUM->SBUF bf16, no max
                                           subtraction (scores are O(5))
  phase 2 (run one window LATE so phase-1 scores/exp of window w+1 keep
  ACT busy while the PE chews through this dense block):
    O+ [65, 1024] += V+.T @ E              bf16, ones row accumulates the
                                           softmax denominator (row 64)
    OC[h*64:(h+1)*64] = O+[0:64] * replicate(1/O+[64])   (DVE reciprocal,
                                           PE outer-product replicate)
    out rows = OC.T @ WoT_c                fp32r partial projection

Scores/AV are bf16 (1 cycle/row on the PE vs 2 for fp32(r), fp32 PSUM
accumulation); the output projection stays fp32r. Measured accuracy
~8e-3 relative to the fp32 reference at the output absmax.
"""

import sys

sys.path.insert(0, "/opt/trn_rl_repo")

import numpy as np

import concourse.bacc as bacc
import concourse.mybir as mybir
import concourse.tile as tile
from concourse.bass_utils import run_bass_kernel_spmd
from concourse.masks import make_identity

F32 = mybir.dt.float32
R = mybir.dt.float32r
BF16 = mybir.dt.bfloat16
EXP = mybir.ActivationFunctionType.Exp

D = 1024
BATCH = 2
SEQ = 2048
M = BATCH * SEQ  # 4096 token rows
HEADS_PER_CORE = 2
DK = 64
HG = HEADS_PER_CORE * DK  # 128-wide head-group slice per core
N_CORES = 8
KT_TILES = D // 128  # 8 contraction tiles for the projections
N_CHUNKS = M // 512  # 8 column chunks of the projected activations
JC = SEQ // 128  # 16 j-chunks per batch
SCALE = 1.0 / np.sqrt(DK)


def build_bass():
    nc = bacc.Bacc(None)

    qT = nc.dram_tensor("qT", [D, M], BF16, kind="ExternalInput")
    kT = nc.dram_tensor("kT", [D, M], BF16, kind="ExternalInput")
    vT = nc.dram_tensor("vT", [D, M], BF16, kind="ExternalInput")
    wqT = nc.dram_tensor("wqT", [D, HG], BF16, kind="ExternalInput")
    wkT = nc.dram_tensor("wkT", [D, HG], BF16, kind="ExternalInput")
    wvT = nc.dram_tensor("wvT", [D, HG], BF16, kind="ExternalInput")
    woT = nc.dram_tensor("woT", [HG, D], R, kind="ExternalInput")
    out = nc.dram_tensor("out", [M, D], F32, kind="ExternalOutput")

    with tile.TileContext(nc) as tc:
        with (
            tc.tile_pool(name="consts", bufs=1) as cst,
            tc.tile_pool(name="acts", bufs=1) as acts,
            tc.tile_pool(name="vp", bufs=1) as vp_pool,
            tc.tile_pool(name="ocpool", bufs=2) as ocpool,
            tc.tile_pool(name="outpool", bufs=2) as outpool,
            tc.tile_pool(name="small", bufs=2) as small,
        ):
            # --- constants ---
            ident_f = cst.tile([128, 128], F32)
            make_identity(nc, ident_f)
            ident = cst.tile([128, 128], R)
            nc.vector.tensor_copy(ident[:], ident_f[:])

            ones_f = cst.tile([128, 1], F32)
            nc.gpsimd.memset(ones_f[:], 1.0)
            onescol = cst.tile([128, 1], BF16)
            nc.vector.tensor_copy(onescol[:], ones_f[:])
            ones64 = cst.tile([1, 64], R)
            nc.vector.tensor_copy(ones64[:], ones_f[0:1, 0:1].to_broadcast([1, 64]))

            # warm the ACT exp table while DMA streams inputs
            scratch = cst.tile([1, 64], F32)
            nc.scalar.activation(
                scratch[:], ones_f[0:1, 0:1].to_broadcast([1, 64]), EXP
            )

            wo_sb = acts.tile([HG, D], R)
            nc.sync.dma_start(wo_sb[:], woT[:])

            QT = acts.tile([HG, M], BF16)
            KT = acts.tile([HG, M], BF16)

            vp_tiles = {}
            with tc.tile_pool(name="vtpool", bufs=1) as vtpool:
                VT = vtpool.tile([HG, M], R)

                # --- projections: TT = W_c @ x.T, feature-major [HG, M] ---
                with (
                    tc.tile_pool(name="wpool", bufs=1) as wpool,
                    tc.tile_pool(name="stage", bufs=3) as stage,
                    tc.tile_pool(name="pp", bufs=1, space="PSUM") as pp,
                ):
                    wq_sb = wpool.tile([128, KT_TILES, HG], BF16)
                    wk_sb = wpool.tile([128, KT_TILES, HG], BF16)
                    wv_sb = wpool.tile([128, KT_TILES, HG], BF16)
                    for w_sb, w_dram in ((wq_sb, wqT), (wk_sb, wkT), (wv_sb, wvT)):
                        nc.sync.dma_start(
                            w_sb[:], w_dram.rearrange("(ko p) n -> p ko n", p=128)
                        )
                    for TT, w_sb, x_dram in (
                        (QT, wq_sb, qT),
                        (KT, wk_sb, kT),
                        (VT, wv_sb, vT),
                    ):
                        pp_tiles = [
                            pp.tile([128, 512], F32, tag=f"pp{n}", name=f"pp{n}")
                            for n in range(N_CHUNKS)
                        ]
                        for k in range(KT_TILES):
                            xst = stage.tile([128, M], BF16, tag="xst")
                            # alternate the two HWDGE queues to keep the
                            # input stream's fixed costs off the critical path
                            eng = nc.sync if k % 2 == 0 else nc.scalar
                            eng.dma_start(
                                xst[:], x_dram[k * 128 : (k + 1) * 128, :]
                            )
                            for n in range(N_CHUNKS):
                                nc.tensor.matmul(
                                    pp_tiles[n][:],
                                    w_sb[:, k, :],
                                    xst[:, n * 512 : (n + 1) * 512],
                                    start=(k == 0),
                                    stop=(k == KT_TILES - 1),
                                )
                        for n in range(N_CHUNKS):
                            nc.vector.tensor_copy(
                                TT[:, n * 512 : (n + 1) * 512], pp_tiles[n][:]
                            )

                # --- V+ tiles: [128 j, 64+1] bf16 per (head, j-chunk) ---
                with tc.tile_pool(name="pst", bufs=2, space="PSUM") as pst:
                    for h in range(HEADS_PER_CORE):
                        hs = slice(h * DK, (h + 1) * DK)
                        id_h = ident[hs, hs]
                        for jg in range(M // 128):
                            tp = pst.tile([128, 64], R, tag="tp")
                            nc.tensor.transpose(
                                tp[:], VT[hs, jg * 128 : (jg + 1) * 128], id_h
                            )
                            vpt = vp_pool.tile(
                                [128, DK + 1], BF16, tag=f"vp_{h}_{jg}"
                            )
                            nc.vector.tensor_copy(vpt[:, 0:DK], tp[:])
                            nc.vector.tensor_copy(vpt[:, DK : DK + 1], onescol[:])
                            vp_tiles[(h, jg)] = vpt
            # VT + projection staging SBUF freed here

            with (
                tc.tile_pool(name="epool", bufs=68) as epool,
                tc.tile_pool(name="psb", bufs=2, space="PSUM") as psb,
                tc.tile_pool(name="pso", bufs=2, space="PSUM") as pso,
            ):
                windows = [(b, ih) for b in range(BATCH) for ih in range(2)]

                def emit_scores(b, ih, jc, e_tiles):
                    i0 = b * SEQ + ih * 1024
                    j0 = b * SEQ + jc * 128
                    for h in range(HEADS_PER_CORE):
                        hs = slice(h * DK, (h + 1) * DK)
                        ps_s = psb.tile([128, 1024], F32, tag="big")
                        for iw in range(2):
                            nc.tensor.matmul(
                                ps_s[:, iw * 512 : (iw + 1) * 512],
                                KT[hs, j0 : j0 + 128],
                                QT[hs, i0 + iw * 512 : i0 + (iw + 1) * 512],
                                start=True,
                                stop=True,
                            )
                        e_t = epool.tile([128, 1024], BF16, tag="e")
                        nc.scalar.activation(e_t[:], ps_s[:], EXP, scale=SCALE)
                        e_tiles[(h, jc)] = e_t

                def emit_av(st, jc):
                    if jc == 0:
                        st["po"] = {
                            h: pso.tile([DK + 1, 1024], F32, tag="po", name=f"po{h}")
                            for h in range(HEADS_PER_CORE)
                        }
                    b, po, e_tiles = st["b"], st["po"], st["e"]
                    jg = b * JC + jc
                    for h in range(HEADS_PER_CORE):
                        for iw in range(2):
                            nc.tensor.matmul(
                                po[h][:, iw * 512 : (iw + 1) * 512],
                                vp_tiles[(h, jg)][:],
                                e_tiles[(h, jc)][:, iw * 512 : (iw + 1) * 512],
                                start=(jc == 0),
                                stop=(jc == JC - 1),
                            )

                def emit_normalize(st):
                    po = st["po"]
                    oc = ocpool.tile([HG, 1024], R, tag="oc")
                    for h in range(HEADS_PER_CORE):
                        hs = slice(h * DK, (h + 1) * DK)
                        rec_row = small.tile([1, 1024], F32, tag="rrow", name=f"rr{h}")
                        nc.vector.reciprocal(rec_row[:], po[h][DK : DK + 1, :])
                        rcr = small.tile([1, 1024], R, tag="rcr", name=f"rcr{h}")
                        nc.vector.tensor_copy(rcr[:], rec_row[:])
                        rep_ps = psb.tile([64, 1024], F32, tag="big")
                        for iw in range(2):
                            nc.tensor.matmul(
                                rep_ps[:, iw * 512 : (iw + 1) * 512],
                                ones64[:],
                                rcr[:, iw * 512 : (iw + 1) * 512],
                                start=True,
                                stop=True,
                            )
                        rec_sb = small.tile([64, 1024], F32, tag="recsb")
                        nc.vector.tensor_copy(rec_sb[:], rep_ps[:])
                        nc.vector.tensor_tensor(
                            oc[hs, :], po[h][0:DK, :], rec_sb[:], mybir.AluOpType.mult
                        )
                    st["oc"] = oc

                def emit_wo(st, ic):
                    b, ih, oc = st["b"], st["ih"], st["oc"]
                    i0 = b * SEQ + ih * 1024
                    wo_ps = psb.tile([128, 1024], F32, tag="big")
                    for oh in range(2):
                        nc.tensor.matmul(
                            wo_ps[:, oh * 512 : (oh + 1) * 512],
                            oc[:, ic * 128 : (ic + 1) * 128],
                            wo_sb[:, oh * 512 : (oh + 1) * 512],
                            start=True,
                            stop=True,
                        )
                    out_sb = outpool.tile([128, 1024], F32, tag="os")
                    nc.vector.tensor_copy(out_sb[:], wo_ps[:])
                    r0 = i0 + ic * 128
                    nc.scalar.dma_start(out[r0 : r0 + 128, :], out_sb[:])

                # 3-stage software pipeline at j-chunk granularity: scores+exp
                # of window w interleave with AV of w-1 and the output
                # projection of w-2, so ACT (exp) never starves while the PE
                # runs the dense AV/Wo blocks
                av_st = None  # window in its AV stage
                wo_st = None  # window in its Wo stage
                for w in windows + [None]:
                    cur = (
                        {"b": w[0], "ih": w[1], "e": {}} if w is not None else None
                    )
                    for jc in range(JC):
                        if cur is not None:
                            emit_scores(cur["b"], cur["ih"], jc, cur["e"])
                        if av_st is not None:
                            emit_av(av_st, jc)
                        if wo_st is not None and jc % 2 == 0:
                            emit_wo(wo_st, jc // 2)
                    if av_st is not None:
                        emit_normalize(av_st)
                    av_st, wo_st = cur, av_st
                # drain: Wo of the last window
                for ic in range(8):
                    emit_wo(wo_st, ic)

    nc.compile()
    return nc


def kernel(q, k, v, Wq, Wk, Wv, Wo):
    import ml_dtypes

    bf = ml_dtypes.bfloat16
    q = np.asarray(q, dtype=np.float32)
    k = np.asarray(k, dtype=np.float32)
    v = np.asarray(v, dtype=np.float32)
    Wq = np.asarray(Wq, dtype=np.float32)
    Wk = np.asarray(Wk, dtype=np.float32)
    Wv = np.asarray(Wv, dtype=np.float32)
    Wo = np.asarray(Wo, dtype=np.float32)

    qT = np.ascontiguousarray(q.reshape(M, D).T.astype(bf))
    kT = np.ascontiguousarray(k.reshape(M, D).T.astype(bf))
    vT = np.ascontiguousarray(v.reshape(M, D).T.astype(bf))

    in_maps = []
    for c in range(N_CORES):
        cs = slice(c * HG, (c + 1) * HG)
        in_maps.append(
            {
                "qT": qT,
                "kT": kT,
                "vT": vT,
                "wqT": np.ascontiguousarray(Wq[cs, :].T.astype(bf)),
                "wkT": np.ascontiguousarray(Wk[cs, :].T.astype(bf)),
                "wvT": np.ascontiguousarray(Wv[cs, :].T.astype(bf)),
                "woT": np.ascontiguousarray(Wo[:, cs].T),
            }
        )

    nc = build_bass()

    def run_once():
        res = run_bass_kernel_spmd(nc, in_maps, core_ids=list(range(N_CORES)))
        acc = res.results[0]["out"].astype(np.float32)
        for c in range(1, N_CORES):
            acc = acc + res.results[c]["out"]
        return acc

    acc = run_once()
    if not np.isfinite(acc).all():
        acc = run_once()  # guard against sporadic device flake
    return acc.reshape(BATCH, SEQ, D)



# revision 7
# speedup vs baseline: 1.4782x; 1.4782x over previous
"""Trainium2 Bass kernel for 16-head MHA (B=2, S=2048, D=1024), fp32 I/O.

Sharding: 2-way batch-parallel x 4-way head-parallel across 8 NeuronCores.
Core c = (b, g) owns batch b and head group g (4 heads, a 256-wide slice of
the QKV projection output and of Wo's input dim). Each core computes its
(batch, head-group)'s full attention plus a partial output projection over
its batch's tokens; the host sums the 4 partials per batch. Versus pure
head-sharding this halves both input DMA (12MB/core) and output DMA (8MB).

Per-core dataflow (feature-major; host pre-transposes and casts to bf16):
  QT/KT [2x 128, 2048] bf16 = W_pair @ x.T    per head-pair tile
  V+ tiles [128j, 65] bf16                    VT j-chunks PE-transposed + ones
  windows = (i-block 512, head-pair t), t-major order, 8 windows:
    S.T pair [128j, 1024] = KT.T @ QT         both heads of the pair land in
                                              opposite PE row-halves and run
                                              CONCURRENTLY (row tiling)
    E = exp(S.T/8)                            one ACT instr per pair
    O+ [65, 512] += V+.T @ E_h                ones row = softmax denominator
    oc = O+[0:64] * replicate(1/O+[64])       DVE + PE outer-product replicate
    out rows = oc.T @ WoT  (fp32r)            partial projection
  K/V/Q projections and Wo are interleaved into the window loop as
  background work so the PE never idles; exp is the only ACT work and all
  DMA dispatch lives on SP (inputs, deadline-ordered) and DVE (outputs).

PSUM budget (8 banks): scores pair tiles 2x2 + AV accumulators 2 + shared
aux (proj/Wo/transpose/replicate) 2.
"""

import sys

sys.path.insert(0, "/opt/trn_rl_repo")

import numpy as np

import concourse.bacc as bacc
import concourse.mybir as mybir
import concourse.tile as tile
from concourse.bass_utils import run_bass_kernel_spmd
from concourse.masks import make_identity

F32 = mybir.dt.float32
R = mybir.dt.float32r
BF16 = mybir.dt.bfloat16
EXP = mybir.ActivationFunctionType.Exp

D = 1024
BATCH = 2
SEQ = 2048
DK = 64
NH = 4  # heads per core
HG = NH * DK  # 256-wide head-group slice per core
NP = 2  # head-pairs per core
N_CORES = 8
KT_TILES = D // 128  # 8 contraction tiles for the projections
NCH = SEQ // 512  # 4 token chunks
JC = SEQ // 128  # 16 j-chunks
IB = 512  # i-block (query window)
NIB = SEQ // IB  # 4
SCALE = 1.0 / np.sqrt(DK)


def build_bass():
    nc = bacc.Bacc(None)

    xq = nc.dram_tensor("xq", [D, SEQ], BF16, kind="ExternalInput")
    xk = nc.dram_tensor("xk", [D, SEQ], BF16, kind="ExternalInput")
    xv = nc.dram_tensor("xv", [D, SEQ], BF16, kind="ExternalInput")
    wq = nc.dram_tensor("wq", [D, HG], BF16, kind="ExternalInput")
    wk = nc.dram_tensor("wk", [D, HG], BF16, kind="ExternalInput")
    wv = nc.dram_tensor("wv", [D, HG], BF16, kind="ExternalInput")
    wo = nc.dram_tensor("wo", [HG, D], R, kind="ExternalInput")
    out = nc.dram_tensor("out", [SEQ, D], F32, kind="ExternalOutput")

    with tile.TileContext(nc) as tc:
        with (
            tc.tile_pool(name="consts", bufs=1) as cst,
            tc.tile_pool(name="wpool", bufs=1) as wpool,
            tc.tile_pool(name="acts", bufs=1) as acts,
            tc.tile_pool(name="vp", bufs=1) as vp_pool,
            tc.tile_pool(name="stage", bufs=1) as stage,
            tc.tile_pool(name="vtc", bufs=2) as vtc,
            tc.tile_pool(name="epool", bufs=20) as epool,
            tc.tile_pool(name="ocpool", bufs=1) as ocpool,
            tc.tile_pool(name="outpool", bufs=2) as outpool,
            tc.tile_pool(name="small", bufs=2) as small,
            tc.tile_pool(name="psb", bufs=2, space="PSUM") as psb,
            tc.tile_pool(name="pso", bufs=2, space="PSUM") as pso,
            tc.tile_pool(name="paux", bufs=2, space="PSUM") as paux,
        ):
            # --- constants ---
            ident_f = cst.tile([128, 128], F32)
            make_identity(nc, ident_f)
            ident = cst.tile([128, 128], R)
            nc.vector.tensor_copy(ident[:], ident_f[:])

            ones_f = cst.tile([128, 1], F32)
            nc.gpsimd.memset(ones_f[:], 1.0)
            onescol = cst.tile([128, 1], BF16)
            nc.vector.tensor_copy(onescol[:], ones_f[:])
            ones64 = cst.tile([1, 64], R)
            nc.vector.tensor_copy(ones64[:], ones_f[0:1, 0:1].to_broadcast([1, 64]))

            # warm the ACT exp table while DMA streams inputs
            scratch = cst.tile([1, 64], F32)
            nc.scalar.activation(
                scratch[:], ones_f[0:1, 0:1].to_broadcast([1, 64]), EXP
            )

            # --- weight tiles ---
            wq_sb = wpool.tile([128, KT_TILES, HG], BF16)
            wk_sb = wpool.tile([128, KT_TILES, HG], BF16)
            wv_sb = wpool.tile([128, KT_TILES, HG], BF16)
            wo_sb = [wpool.tile([128, D], R, name=f"wo{t}") for t in range(NP)]

            # --- activation tiles ---
            QT = [acts.tile([128, SEQ], BF16, name=f"QT{t}") for t in range(NP)]
            KT = [acts.tile([128, SEQ], BF16, name=f"KT{t}") for t in range(NP)]

            # --- input staging (chunks held until their pair-1 proj reads) ---
            xst = {}  # (tensor_key, chunk) -> tile
            x_r = {
                "q": xq.rearrange("(ko p) n -> p ko n", p=128),
                "k": xk.rearrange("(ko p) n -> p ko n", p=128),
                "v": xv.rearrange("(ko p) n -> p ko n", p=128),
            }

            def dma_chunk(key, c, bufs):
                t = stage.tile(
                    [128, KT_TILES, 512], BF16, tag=f"x{key}", bufs=bufs,
                    name=f"x{key}{c}",
                )
                nc.sync.dma_start(t[:], x_r[key][:, :, c * 512 : (c + 1) * 512])
                xst[(key, c)] = t

            # input DMA stream on SP, deadline order. k/q chunks are consumed
            # by both head-pairs' projections adjacently (bufs=2); v chunks
            # stay staged until the deferred pair-1 V projection (bufs=4).
            nc.sync.dma_start(wk_sb[:], wk.rearrange("(ko p) n -> p ko n", p=128))
            dma_chunk("k", 0, 2)
            nc.sync.dma_start(wq_sb[:], wq.rearrange("(ko p) n -> p ko n", p=128))
            dma_chunk("q", 0, 2)
            dma_chunk("k", 1, 2)
            dma_chunk("q", 1, 2)
            dma_chunk("k", 2, 2)
            dma_chunk("k", 3, 2)
            nc.sync.dma_start(wv_sb[:], wv.rearrange("(ko p) n -> p ko n", p=128))
            dma_chunk("v", 0, 4)
            dma_chunk("v", 1, 4)
            dma_chunk("v", 2, 4)
            dma_chunk("v", 3, 4)
            dma_chunk("q", 2, 2)
            dma_chunk("q", 3, 2)
            for t in range(NP):
                nc.sync.dma_start(wo_sb[t][:], wo[t * 128 : (t + 1) * 128, :])

            w_sb = {"q": wq_sb, "k": wk_sb, "v": wv_sb}
            vp_tiles = {}  # (h, jg) -> V+ tile
            vt_chunks = {}  # (t, c) -> VT chunk tile (fp32r)

            def proj_group(key, c, t):
                """Project x[key] chunk c for head-pair t."""
                pp = paux.tile([128, 512], F32, tag="aux", name=f"pp_{key}{c}{t}")
                for k in range(KT_TILES):
                    nc.tensor.matmul(
                        pp[:],
                        w_sb[key][:, k, t * 128 : (t + 1) * 128],
                        xst[(key, c)][:, k, :],
                        start=(k == 0),
                        stop=(k == KT_TILES - 1),
                    )
                if key == "v":
                    vt = vtc.tile([128, 512], R, tag="vt", name=f"vt{t}{c}")
                    nc.vector.tensor_copy(vt[:], pp[:])
                    vt_chunks[(t, c)] = vt
                else:
                    TT = QT[t] if key == "q" else KT[t]
                    nc.vector.tensor_copy(TT[:, c * 512 : (c + 1) * 512], pp[:])

            def vplus(t, jgs):
                """Build V+ tiles for both heads of pair t, j-groups jgs."""
                for jg in jgs:
                    vt = vt_chunks[(t, jg // 4)]
                    jl = (jg % 4) * 128
                    for r in range(2):
                        h = 2 * t + r
                        hs = slice(r * 64, (r + 1) * 64)
                        tp = paux.tile([128, 64], R, tag="aux", name=f"tp{h}{jg}")
                        nc.tensor.transpose(
                            tp[:], vt[hs, jl : jl + 128], ident[hs, hs]
                        )
                        vpt = vp_pool.tile(
                            [128, DK + 1], BF16, tag=f"vp_{h}_{jg}", name=f"vp{h}{jg}"
                        )
                        nc.vector.tensor_copy(vpt[:, 0:DK], tp[:])
                        nc.vector.tensor_copy(vpt[:, DK : DK + 1], onescol[:])
                        vp_tiles[(h, jg)] = vpt

            ocs = {}  # (c, t) -> oc tile [128, 512] fp32r

            def emit_normalize(st):
                c, t, po = st["c"], st["t"], st["po"]
                oc = ocpool.tile(
                    [128, IB], R, tag=f"oc{t}", bufs=(4 if t == 0 else 2),
                    name=f"oc{c}{t}",
                )
                for r in range(2):
                    rrow = small.tile([1, IB], F32, tag="rrow", name=f"rr{r}")
                    nc.vector.reciprocal(rrow[:], po[r][DK : DK + 1, :])
                    rcr = small.tile([1, IB], R, tag="rcr", name=f"rcr{r}")
                    nc.vector.tensor_copy(rcr[:], rrow[:])
                    rep = paux.tile([64, IB], F32, tag="aux", name=f"rep{r}")
                    nc.tensor.matmul(rep[:], ones64[:], rcr[:], start=True, stop=True)
                    rec_sb = small.tile([64, IB], F32, tag="recsb", name=f"rs{r}")
                    nc.vector.tensor_copy(rec_sb[:], rep[:])
                    nc.vector.tensor_tensor(
                        oc[r * 64 : (r + 1) * 64, :],
                        po[r][0:DK, :],
                        rec_sb[:],
                        mybir.AluOpType.mult,
                    )
                ocs[(c, t)] = oc

            def wo_piece(c, tt):
                """Output projection for token tile tt of i-block c."""
                os_t = outpool.tile([128, D], F32, tag="os", name=f"os{c}{tt}")
                for oh in range(2):
                    wo_ps = paux.tile(
                        [128, 512], F32, tag="aux", name=f"wops{c}{tt}{oh}"
                    )
                    for t in range(NP):
                        nc.tensor.matmul(
                            wo_ps[:],
                            ocs[(c, t)][:, tt * 128 : (tt + 1) * 128],
                            wo_sb[t][:, oh * 512 : (oh + 1) * 512],
                            start=(t == 0),
                            stop=(t == NP - 1),
                        )
                    nc.vector.tensor_copy(
                        os_t[:, oh * 512 : (oh + 1) * 512], wo_ps[:]
                    )
                r0 = c * IB + tt * 128
                nc.sync.dma_start(out[r0 : r0 + 128, :], os_t[:])

            def emit_scores(st, jc):
                c, t = st["c"], st["t"]
                i0 = c * IB
                ps = psb.tile([128, 2 * IB], F32, tag="ps", name=f"ps{jc}")
                for r in range(2):
                    hs = slice(r * 64, (r + 1) * 64)
                    nc.tensor.matmul(
                        ps[:, r * IB : (r + 1) * IB],
                        KT[t][hs, jc * 128 : (jc + 1) * 128],
                        QT[t][hs, i0 : i0 + IB],
                        start=True,
                        stop=True,
                    )
                e_t = epool.tile([128, 2 * IB], BF16, tag="e", name=f"e{jc}")
                nc.scalar.activation(e_t[:], ps[:], EXP, scale=SCALE)
                st["e"][jc] = e_t

            def emit_av(st, jc):
                t = st["t"]
                if jc == 0:
                    st["po"] = {
                        r: pso.tile([DK + 1, IB], F32, tag="po", name=f"po{r}")
                        for r in range(2)
                    }
                po = st["po"]
                for r in range(2):
                    nc.tensor.matmul(
                        po[r][:],
                        vp_tiles[(2 * t + r, jc)][:],
                        st["e"][jc][:, r * IB : (r + 1) * IB],
                        start=(jc == 0),
                        stop=(jc == JC - 1),
                    )

            # --- prefix: chunk-0 K/Q for both pairs ---
            proj_group("k", 0, 0)
            proj_group("k", 0, 1)
            proj_group("q", 0, 0)
            proj_group("q", 0, 1)

            # --- background work plan (t-major window order) ---
            bg = {
                0: [
                    lambda: proj_group("k", 1, 0),
                    lambda: proj_group("k", 1, 1),
                    lambda: proj_group("q", 1, 0),
                    lambda: proj_group("q", 1, 1),
                    lambda: proj_group("k", 2, 0),
                    lambda: proj_group("k", 2, 1),
                    lambda: proj_group("k", 3, 0),
                    lambda: proj_group("k", 3, 1),
                    lambda: proj_group("v", 0, 0),
                    lambda: vplus(0, [0, 1]),
                    lambda: vplus(0, [2, 3]),
                ],
                1: [
                    lambda: proj_group("v", 1, 0),
                    lambda: vplus(0, [4, 5]),
                    lambda: vplus(0, [6, 7]),
                    lambda: proj_group("v", 2, 0),
                    lambda: vplus(0, [8, 9]),
                    lambda: vplus(0, [10, 11]),
                    lambda: proj_group("v", 3, 0),
                    lambda: vplus(0, [12, 13]),
                    lambda: vplus(0, [14, 15]),
                    lambda: proj_group("q", 2, 0),
                    lambda: proj_group("q", 2, 1),
                ],
                2: [
                    lambda: proj_group("q", 3, 0),
                    lambda: proj_group("q", 3, 1),
                    lambda: proj_group("v", 0, 1),
                    lambda: vplus(1, [0, 1]),
                    lambda: vplus(1, [2, 3]),
                ],
                3: [
                    lambda: proj_group("v", 1, 1),
                    lambda: vplus(1, [4, 5]),
                    lambda: vplus(1, [6, 7]),
                    lambda: proj_group("v", 2, 1),
                    lambda: vplus(1, [8, 9]),
                    lambda: vplus(1, [10, 11]),
                ],
                4: [
                    lambda: proj_group("v", 3, 1),
                    lambda: vplus(1, [12, 13]),
                    lambda: vplus(1, [14, 15]),
                ],
                6: [lambda tt=tt: wo_piece(0, tt) for tt in range(4)],
                7: [lambda tt=tt: wo_piece(1, tt) for tt in range(4)],
                8: [lambda tt=tt: wo_piece(2, tt) for tt in range(4)],
                9: [lambda tt=tt: wo_piece(3, tt) for tt in range(4)],
            }

            windows = [(c, t) for t in range(NP) for c in range(NIB)]
            av_st = None
            for idx in range(len(windows) + 2):
                w = windows[idx] if idx < len(windows) else None
                cur = {"c": w[0], "t": w[1], "e": {}} if w is not None else None
                items = bg.get(idx, [])
                done = 0
                for jc in range(JC):
                    want = (jc + 1) * len(items) // JC
                    while done < want:
                        items[done]()
                        done += 1
                    if cur is not None:
                        emit_scores(cur, jc)
                    if av_st is not None:
                        emit_av(av_st, jc)
                if av_st is not None:
                    emit_normalize(av_st)
                av_st = cur

    nc.compile()
    return nc


def build_in_maps(q, k, v, Wq, Wk, Wv, Wo):
    import ml_dtypes

    bf = ml_dtypes.bfloat16
    q = np.asarray(q, dtype=np.float32)
    k = np.asarray(k, dtype=np.float32)
    v = np.asarray(v, dtype=np.float32)
    Wq = np.asarray(Wq, dtype=np.float32)
    Wk = np.asarray(Wk, dtype=np.float32)
    Wv = np.asarray(Wv, dtype=np.float32)
    Wo = np.asarray(Wo, dtype=np.float32)

    in_maps = []
    for b in range(BATCH):
        qT = np.ascontiguousarray(q[b].T.astype(bf))
        kT = np.ascontiguousarray(k[b].T.astype(bf))
        vT = np.ascontiguousarray(v[b].T.astype(bf))
        for g in range(4):
            sl = slice(g * HG, (g + 1) * HG)
            in_maps.append(
                {
                    "xq": qT,
                    "xk": kT,
                    "xv": vT,
                    "wq": np.ascontiguousarray(Wq[sl, :].T.astype(bf)),
                    "wk": np.ascontiguousarray(Wk[sl, :].T.astype(bf)),
                    "wv": np.ascontiguousarray(Wv[sl, :].T.astype(bf)),
                    "wo": np.ascontiguousarray(Wo[:, sl].T),
                }
            )
    return in_maps


def combine_results(results):
    acc = []
    for b in range(BATCH):
        o = results[b * 4]["out"].astype(np.float32)
        for g in range(1, 4):
            o = o + results[b * 4 + g]["out"]
        acc.append(o)
    return np.stack(acc).reshape(BATCH, SEQ, D)


def kernel(q, k, v, Wq, Wk, Wv, Wo):
    in_maps = build_in_maps(q, k, v, Wq, Wk, Wv, Wo)
    nc = build_bass()

    def run_once():
        res = run_bass_kernel_spmd(nc, in_maps, core_ids=list(range(N_CORES)))
        return combine_results(res.results)

    acc = run_once()
    if not np.isfinite(acc).all():
        acc = run_once()  # guard against sporadic device flake
    return acc
